# revision 8
# baseline (speedup 1.0000x reference)
"""CLIP-style contrastive (HCL) loss for B=4096, f32 logits on 8 trn2 cores.

Math reduction (BETA=1, t=0.5, tau+=0.1):
  - imp == neg, so reweight_neg = sum(neg^2) * N / sum(neg).
  - Row i and row i+B of the 2Bx2B sim matrix hold identical value multisets
    (both are {row_i(L), col_i(L)} minus two copies of L[i,i]), so
    loss[i] == loss[i+B] and the mean over 2B rows == mean over B rows.
  - Everything reduces to row sums + col sums of E = exp(2L) and E2 = exp(4L),
    plus the diagonal of L.

Device work per core (rows k*512..(k+1)*512 of L, cast to bf16 on host):
  - 10 pieces in column-major order (all 4 row tiles of the left half, then
    the right half), with a small first piece (early ACT start) and a small
    last piece (short post-ACT drain):
    ACT exp(2x)->bf16 E with fused fp32 row-sum,
    DVE scalar_tensor_tensor E*E->bf16 E2 with fused fp32 row-sum,
    PE ones-matmul per-column sums into PSUM.
  - After the last row tile of each column half, that half's colsums are
    final: evict PSUM->SBUF and DMA. All outputs live in ONE SBUF tile so
    the tail needs a single descriptor-gen (~0.6us) instead of four.
Host: assemble sums, per-row loss formula over 4096 rows in f64, mean.
"""

import os

import numpy as np
import ml_dtypes

import concourse.bacc as bacc
import concourse.bass as bass
import concourse.tile as tile
from concourse import mybir
from concourse.bass_utils import run_bass_kernel_spmd

B = 4096
N_CORES = 8
ROWS_PER_CORE = B // N_CORES  # 512
P = 128
TILES = ROWS_PER_CORE // P  # 4
HALF = B // 2  # 2048 cols per half
M = B // P  # 32 column chunks of 128

TAU_PLUS = 0.1
TEMPERATURE = 0.5
EPS = 1e-8

LPOOL_BUFS = int(os.environ.get("KERNEL_LPOOL_BUFS", "3"))

_NC = None
LAST_RESULTS = None  # BassKernelResults of the most recent run (for test harness)

# (row_tile, col_start, col_len), column-major: finish all row tiles of the
# left half first so its PSUM colsums can ship while the right half computes.
# Small piece 0 starts ACT as early as possible; small piece 9 keeps the
# post-ACT DVE/PE drain short.
PIECES = [
    (0, 0, 512),
    (0, 512, 1536),
    (1, 0, 2048),
    (2, 0, 2048),
    (3, 0, 2048),
    (0, HALF, 2048),
    (1, HALF, 2048),
    (2, HALF, 2048),
    (3, HALF, 1536),
    (3, HALF + 1536, 512),
]
NPIECE = len(PIECES)  # 10

# Output layout in one [128, 84] fp32 tensor:
#   0:16   colsum E,  left half  (chunk-stationary: colsum[m*128+j] = out[j, m])
#   16:32  colsum E2, left half
#   32:42  rowsum E partial per piece
#   42:52  rowsum E2 partial per piece
#   52:68  colsum E,  right half
#   68:84  colsum E2, right half
RS_E = 32
RS_E2 = 42
NCOL = 84


def _build_bass():
    in_dt = mybir.dt.bfloat16
    edt = mybir.dt.bfloat16

    nc = bacc.Bacc(None)
    slab = nc.declare_dram_parameter("slab", [ROWS_PER_CORE, B], in_dt, isOutput=False)
    sums = nc.declare_dram_parameter("sums", [P, NCOL], mybir.dt.float32, isOutput=True)

    with tile.TileContext(nc) as tc:
        with (
            tc.tile_pool(name="lpool", bufs=LPOOL_BUFS) as lpool,
            tc.tile_pool(name="epool", bufs=3) as epool,
            tc.tile_pool(name="e2pool", bufs=3) as e2pool,
            tc.tile_pool(name="singles", bufs=1) as singles,
            tc.tile_pool(name="psum", bufs=1, space="PSUM") as psum_pool,
        ):
            # Build the matmul ones-vector with a GpSimd memset (idle engine)
            # instead of a const-tensor DMA, which sat on the setup path.
            ones = singles.tile([P, 1], mybir.dt.bfloat16)
            nc.gpsimd.memset(ones, 1.0)
            cs = singles.tile([P, NCOL], mybir.dt.float32)
            psE = psum_pool.tile([P, M], mybir.dt.float32)
            psE2 = psum_pool.tile([P, M], mybir.dt.float32)

            for i, (t, c0, clen) in enumerate(PIECES):
                rows = slice(t * P, (t + 1) * P)
                cols = slice(c0, c0 + clen)

                ltile = lpool.tile([P, clen], in_dt, tag=f"ltile{clen}")
                nc.sync.dma_start(out=ltile, in_=slab[rows, cols])

                etile = epool.tile([P, clen], edt, tag=f"etile{clen}")
                nc.scalar.activation(
                    out=etile,
                    in_=ltile,
                    func=mybir.ActivationFunctionType.Exp,
                    scale=2.0,
                    accum_out=cs[:, RS_E + i : RS_E + i + 1],
                )
                e2tile = e2pool.tile([P, clen], edt, tag=f"e2tile{clen}")
                # E2 = (E * 1) * E on DVE, with fused fp32 row-sum.
                nc.vector.scalar_tensor_tensor(
                    out=e2tile,
                    in0=etile,
                    scalar=1.0,
                    in1=etile,
                    op0=mybir.AluOpType.mult,
                    op1=mybir.AluOpType.mult,
                    accum_out=cs[:, RS_E2 + i : RS_E2 + i + 1],
                )

                # PSUM start_tensor_calc zeroes the whole 2KB (partition, bank)
                # zero-region lazily: only the FIRST matmul touching each psum
                # tensor may carry start=True; later writes to still-pending
                # bytes replace (i.e. add to zero), writes to touched bytes
                # accumulate. One start per tensor, ever.
                for m in range(clen // P):
                    gm = c0 // P + m
                    lsl = slice(m * P, (m + 1) * P)
                    first = i == 0 and m == 0
                    last = i == NPIECE - 1 and m == clen // P - 1
                    nc.tensor.matmul(
                        psE[:, gm : gm + 1],
                        etile[:, lsl],
                        ones,
                        start=first,
                        stop=last,
                        skip_group_check=True,
                    )
                    nc.tensor.matmul(
                        psE2[:, gm : gm + 1],
                        e2tile[:, lsl],
                        ones,
                        start=first,
                        stop=last,
                        skip_group_check=True,
                    )

                if t == TILES - 1 and c0 + clen == HALF:
                    # Left half colsums final: evict and ship (one DMA, off
                    # the critical path while the right half computes).
                    nc.vector.tensor_copy(cs[:, 0:16], psE[:, 0:16])
                    nc.scalar.copy(cs[:, 16:32], psE2[:, 0:16])
                    nc.sync.dma_start(out=sums[:, 0:32], in_=cs[:, 0:32])
                elif t == TILES - 1 and c0 + clen == B:
                    # Right half final: evict, then ship rowsums + right-half
                    # colsums in a single DMA (one descriptor-gen on the tail).
                    nc.vector.tensor_copy(cs[:, 52:68], psE[:, 16:32])
                    nc.scalar.copy(cs[:, 68:84], psE2[:, 16:32])
                    nc.sync.dma_start(out=sums[:, 32:NCOL], in_=cs[:, 32:NCOL])
    # Bacc defers register allocation and sync-wait splitting to finalize();
    # run_bass_via_pjrt does not call it, so do it here.
    nc.finalize()
    return nc


def _get_nc():
    global _NC
    if _NC is None:
        _NC = _build_bass()
    return _NC


def kernel(logits: np.ndarray) -> np.ndarray:
    global LAST_RESULTS
    logits = np.ascontiguousarray(np.asarray(logits, dtype=np.float32))
    assert logits.shape == (B, B)

    nc = _get_nc()
    cast = lambda a: np.ascontiguousarray(a.astype(ml_dtypes.bfloat16))
    in_maps = [
        {"slab": cast(logits[k * ROWS_PER_CORE : (k + 1) * ROWS_PER_CORE, :])}
        for k in range(N_CORES)
    ]
    res = run_bass_kernel_spmd(
        nc,
        in_maps,
        core_ids=list(range(N_CORES)),
        trace=bool(int(os.environ.get("KERNEL_TRACE", "0"))),
    )
    LAST_RESULTS = res

    rowsum_E = np.empty(B, dtype=np.float64)
    rowsum_E2 = np.empty(B, dtype=np.float64)
    colsum_E = np.zeros(B, dtype=np.float64)
    colsum_E2 = np.zeros(B, dtype=np.float64)
    for k in range(N_CORES):
        out = res.results[k]["sums"].astype(np.float64)  # [128, 84]
        sl = slice(k * ROWS_PER_CORE, (k + 1) * ROWS_PER_CORE)
        rsE = np.zeros((P, TILES))
        rsE2 = np.zeros((P, TILES))
        for i, (t, _, _) in enumerate(PIECES):
            rsE[:, t] += out[:, RS_E + i]
            rsE2[:, t] += out[:, RS_E2 + i]
        rowsum_E[sl] = rsE.T.reshape(-1)
        rowsum_E2[sl] = rsE2.T.reshape(-1)
        colsum_E[0:HALF] += out[:, 0:16].T.reshape(-1)
        colsum_E2[0:HALF] += out[:, 16:32].T.reshape(-1)
        colsum_E[HALF:B] += out[:, 52:68].T.reshape(-1)
        colsum_E2[HALF:B] += out[:, 68:84].T.reshape(-1)

    d = np.diagonal(logits)
    pos = np.exp(d.astype(np.float64) / TEMPERATURE)
    # The device sums contain exp of the bf16-rounded diagonal; subtract
    # exactly what the device added.
    dD = d.astype(ml_dtypes.bfloat16).astype(np.float64)
    posD = np.exp(dD / TEMPERATURE)
    N = 2 * B - 2
    S1 = rowsum_E + colsum_E - 2.0 * posD
    S2 = rowsum_E2 + colsum_E2 - 2.0 * posD * posD
    reweight = S2 * N / S1
    Ng = (-TAU_PLUS * N * pos + reweight) / (1.0 - TAU_PLUS)
    Ng = np.maximum(Ng, N * np.exp(-1.0 / TEMPERATURE))
    loss = -np.log(pos / (pos + Ng + EPS))
    return np.float32(loss.mean())


# revision 10
# speedup vs baseline: 1.0060x; 1.0060x over previous
"""CLIP-style contrastive (HCL) loss for B=4096, f32 logits on 8 trn2 cores.

Math reduction (BETA=1, t=0.5, tau+=0.1):
  - imp == neg, so reweight_neg = sum(neg^2) * N / sum(neg).
  - Row i and row i+B of the 2Bx2B sim matrix hold identical value multisets
    (both are {row_i(L), col_i(L)} minus two copies of L[i,i]), so
    loss[i] == loss[i+B] and the mean over 2B rows == mean over B rows.
  - Everything reduces to row sums + col sums of E = exp(2L) and E2 = exp(4L),
    plus the diagonal of L.

Device work per core (rows k*512..(k+1)*512 of L, cast to bf16 on host):
  - 10 pieces in column-major order (all 4 row tiles of the left half, then
    the right half), with a small first piece (early ACT start) and a small
    last piece (short post-ACT drain):
    ACT exp(2x)->bf16 E with fused fp32 row-sum,
    DVE scalar_tensor_tensor E*E->bf16 E2 with fused fp32 row-sum,
    PE ones-matmul per-column sums into PSUM.
  - After the last row tile of each column half, that half's colsums are
    final: evict PSUM->SBUF and DMA. All outputs live in ONE SBUF tile so
    the tail needs a single descriptor-gen (~0.6us) instead of four.
Host: assemble sums, per-row loss formula over 4096 rows in f64, mean.
"""

import os

import numpy as np
import ml_dtypes

import concourse.bacc as bacc
import concourse.bass as bass
import concourse.tile as tile
from concourse import mybir
from concourse.bass_utils import run_bass_kernel_spmd

B = 4096
N_CORES = 8
ROWS_PER_CORE = B // N_CORES  # 512
P = 128
TILES = ROWS_PER_CORE // P  # 4
HALF = B // 2  # 2048 cols per half
M = B // P  # 32 column chunks of 128

TAU_PLUS = 0.1
TEMPERATURE = 0.5
EPS = 1e-8

LPOOL_BUFS = int(os.environ.get("KERNEL_LPOOL_BUFS", "3"))

_NC = None
LAST_RESULTS = None  # BassKernelResults of the most recent run (for test harness)

# (row_tile, col_start, col_len), column-major: finish all row tiles of the
# left half first so its PSUM colsums can ship while the right half computes.
# Small piece 0 starts ACT as early as possible; small piece 9 keeps the
# post-ACT DVE/PE drain short.
# All pieces [128, 2048]: narrower pieces emit 1-3KB DMA descriptors, and a
# descriptor costs ~155ns on its queue regardless of payload size, so narrow
# pieces tank DMA throughput and stall ACT (measured: +1us vs this layout).
PIECES = [
    (0, 0, 2048),
    (1, 0, 2048),
    (2, 0, 2048),
    (3, 0, 2048),
    (0, HALF, 2048),
    (1, HALF, 2048),
    (2, HALF, 2048),
    (3, HALF, 2048),
]
NPIECE = len(PIECES)  # 8

# Output layout in one [128, 80] fp32 tensor:
#   0:16   colsum E,  left half  (chunk-stationary: colsum[m*128+j] = out[j, m])
#   16:32  colsum E2, left half
#   32:40  rowsum E partial per piece
#   40:48  rowsum E2 partial per piece
#   48:64  colsum E,  right half
#   64:80  colsum E2, right half
RS_E = 32
RS_E2 = 40
NCOL = 80


def _build_bass():
    in_dt = mybir.dt.bfloat16
    edt = mybir.dt.bfloat16

    nc = bacc.Bacc(None)
    slab = nc.declare_dram_parameter("slab", [ROWS_PER_CORE, B], in_dt, isOutput=False)
    sums = nc.declare_dram_parameter("sums", [P, NCOL], mybir.dt.float32, isOutput=True)

    with tile.TileContext(nc) as tc:
        with (
            tc.tile_pool(name="lpool", bufs=LPOOL_BUFS) as lpool,
            tc.tile_pool(name="epool", bufs=3) as epool,
            tc.tile_pool(name="e2pool", bufs=3) as e2pool,
            tc.tile_pool(name="singles", bufs=1) as singles,
            tc.tile_pool(name="psum", bufs=1, space="PSUM") as psum_pool,
        ):
            # Build the matmul ones-vector with a GpSimd memset (idle engine)
            # instead of a const-tensor DMA, which sat on the setup path.
            ones = singles.tile([P, 1], mybir.dt.bfloat16)
            nc.gpsimd.memset(ones, 1.0)
            cs = singles.tile([P, NCOL], mybir.dt.float32)
            psE = psum_pool.tile([P, M], mybir.dt.float32)
            psE2 = psum_pool.tile([P, M], mybir.dt.float32)

            for i, (t, c0, clen) in enumerate(PIECES):
                rows = slice(t * P, (t + 1) * P)
                cols = slice(c0, c0 + clen)

                ltile = lpool.tile([P, clen], in_dt, tag=f"ltile{clen}")
                nc.sync.dma_start(out=ltile, in_=slab[rows, cols])

                etile = epool.tile([P, clen], edt, tag=f"etile{clen}")
                nc.scalar.activation(
                    out=etile,
                    in_=ltile,
                    func=mybir.ActivationFunctionType.Exp,
                    scale=2.0,
                    accum_out=cs[:, RS_E + i : RS_E + i + 1],
                )
                e2tile = e2pool.tile([P, clen], edt, tag=f"e2tile{clen}")
                # E2 = (E * 1) * E on DVE, with fused fp32 row-sum.
                nc.vector.scalar_tensor_tensor(
                    out=e2tile,
                    in0=etile,
                    scalar=1.0,
                    in1=etile,
                    op0=mybir.AluOpType.mult,
                    op1=mybir.AluOpType.mult,
                    accum_out=cs[:, RS_E2 + i : RS_E2 + i + 1],
                )

                # PSUM start_tensor_calc zeroes the whole 2KB (partition, bank)
                # zero-region lazily: only the FIRST matmul touching each psum
                # tensor may carry start=True; later writes to still-pending
                # bytes replace (i.e. add to zero), writes to touched bytes
                # accumulate. One start per tensor, ever.
                for m in range(clen // P):
                    gm = c0 // P + m
                    lsl = slice(m * P, (m + 1) * P)
                    first = i == 0 and m == 0
                    last = i == NPIECE - 1 and m == clen // P - 1
                    nc.tensor.matmul(
                        psE[:, gm : gm + 1],
                        etile[:, lsl],
                        ones,
                        start=first,
                        stop=last,
                        skip_group_check=True,
                    )
                    nc.tensor.matmul(
                        psE2[:, gm : gm + 1],
                        e2tile[:, lsl],
                        ones,
                        start=first,
                        stop=last,
                        skip_group_check=True,
                    )

                if t == TILES - 1 and c0 + clen == HALF:
                    # Left half colsums final: evict and ship (one DMA, off
                    # the critical path while the right half computes).
                    nc.vector.tensor_copy(cs[:, 0:16], psE[:, 0:16])
                    nc.scalar.copy(cs[:, 16:32], psE2[:, 0:16])
                    nc.sync.dma_start(out=sums[:, 0:32], in_=cs[:, 0:32])
                elif t == TILES - 1 and c0 + clen == B:
                    # Right half final: evict, then ship rowsums + right-half
                    # colsums in a single DMA (one descriptor-gen on the tail).
                    nc.vector.tensor_copy(cs[:, 48:64], psE[:, 16:32])
                    nc.scalar.copy(cs[:, 64:80], psE2[:, 16:32])
                    nc.sync.dma_start(out=sums[:, 32:NCOL], in_=cs[:, 32:NCOL])
    # Bacc defers register allocation and sync-wait splitting to finalize();
    # run_bass_via_pjrt does not call it, so do it here.
    nc.finalize()
    return nc


def _get_nc():
    global _NC
    if _NC is None:
        _NC = _build_bass()
    return _NC


def kernel(logits: np.ndarray) -> np.ndarray:
    global LAST_RESULTS
    logits = np.ascontiguousarray(np.asarray(logits, dtype=np.float32))
    assert logits.shape == (B, B)

    nc = _get_nc()
    cast = lambda a: np.ascontiguousarray(a.astype(ml_dtypes.bfloat16))
    in_maps = [
        {"slab": cast(logits[k * ROWS_PER_CORE : (k + 1) * ROWS_PER_CORE, :])}
        for k in range(N_CORES)
    ]
    res = run_bass_kernel_spmd(
        nc,
        in_maps,
        core_ids=list(range(N_CORES)),
        trace=bool(int(os.environ.get("KERNEL_TRACE", "0"))),
    )
    LAST_RESULTS = res

    rowsum_E = np.empty(B, dtype=np.float64)
    rowsum_E2 = np.empty(B, dtype=np.float64)
    colsum_E = np.zeros(B, dtype=np.float64)
    colsum_E2 = np.zeros(B, dtype=np.float64)
    for k in range(N_CORES):
        out = res.results[k]["sums"].astype(np.float64)  # [128, 80]
        sl = slice(k * ROWS_PER_CORE, (k + 1) * ROWS_PER_CORE)
        rsE = np.zeros((P, TILES))
        rsE2 = np.zeros((P, TILES))
        for i, (t, _, _) in enumerate(PIECES):
            rsE[:, t] += out[:, RS_E + i]
            rsE2[:, t] += out[:, RS_E2 + i]
        rowsum_E[sl] = rsE.T.reshape(-1)
        rowsum_E2[sl] = rsE2.T.reshape(-1)
        colsum_E[0:HALF] += out[:, 0:16].T.reshape(-1)
        colsum_E2[0:HALF] += out[:, 16:32].T.reshape(-1)
        colsum_E[HALF:B] += out[:, 48:64].T.reshape(-1)
        colsum_E2[HALF:B] += out[:, 64:80].T.reshape(-1)

    d = np.diagonal(logits)
    pos = np.exp(d.astype(np.float64) / TEMPERATURE)
    # The device sums contain exp of the bf16-rounded diagonal; subtract
    # exactly what the device added.
    dD = d.astype(ml_dtypes.bfloat16).astype(np.float64)
    posD = np.exp(dD / TEMPERATURE)
    N = 2 * B - 2
    S1 = rowsum_E + colsum_E - 2.0 * posD
    S2 = rowsum_E2 + colsum_E2 - 2.0 * posD * posD
    reweight = S2 * N / S1
    Ng = (-TAU_PLUS * N * pos + reweight) / (1.0 - TAU_PLUS)
    Ng = np.maximum(Ng, N * np.exp(-1.0 / TEMPERATURE))
    loss = -np.log(pos / (pos + Ng + EPS))
    return np.float32(loss.mean())
